# revision 12
# baseline (speedup 1.0000x reference)
"""GCN 2-layer + global_add_pool + linear on 8 TRN2 NeuronCores.

Strategy (data-parallel over dst-node ranges):
  - core c owns dst nodes [c*1250, (c+1)*1250)
  - host precomputes, per core: gather indices (edge srcs sorted by dst,
    padded per 128-dst tile to a uniform chunk count) and a block one-hot
    "segment sum" matrix S with the GCN edge norms as values.
  - device: XW1 = x_c @ W1 (bf16 PE) -> AllGather -> dma_gather messages ->
    S-matmul segmented sum (PE, accumulating in PSUM f32) -> relu -> H1;
    transpose H1 tiles on PE -> XW2 = H1 @ W2 -> AllGather -> gather ->
    S-matmul -> relu -> H2; pooling via per-tile graph-mask matmul ->
    AllReduce -> out^T = Wlin^T @ pooled^T.
  - biases are zeros in this problem's setup; b1/b2 get a DVE add only if
    nonzero, blin is added on the host (linear post-op).
"""
import os
import numpy as np
import ml_dtypes

import concourse.bass as bass
import concourse.mybir as mybir
import concourse.tile as tile
from concourse import bacc
from concourse.bass_utils import run_bass_kernel_spmd
from concourse.masks import make_identity

N = 10000
E = 160000
F = 512          # IN_F == CL1 == CL2 == 512
OUT_F = 128
G = 64
NCORES = 8
NPC = N // NCORES          # 1250 nodes per core
NPAD = 1280                # padded to 10 tiles of 128
NT = NPAD // 128           # 10 node tiles per core
BF16 = mybir.dt.bfloat16
F32 = mybir.dt.float32


def _fix_drain_waits(nc, max_waits=1):
    """walrus CTRL instrs (Drain) accept only 1 sync wait; split extras."""
    for fn in nc.m.functions:
        for bb in fn.blocks:
            insts = bb.instructions
            i = 0
            while i < len(insts):
                inst = insts[i]
                si = inst.sync_info
                if si is not None and si.on_wait and len(si.on_wait) > max_waits:
                    waits = list(si.on_wait)
                    chunks = [waits[j:j + max_waits]
                              for j in range(0, len(waits), max_waits)]
                    inst.sync_info = mybir.SyncInfo(
                        on_wait=chunks[-1], on_update=list(si.on_update))
                    for k, ch in enumerate(chunks[:-1]):
                        d = mybir.InstDrain(
                            name=f"{inst.name}_ws{k}",
                            engine=inst.engine,
                            ins=[], outs=[],
                            sync_info=mybir.SyncInfo(on_wait=ch, on_update=[]),
                        )
                        insts.insert(i + k, d)
                    i += len(chunks) - 1
                i += 1


def build(n_chunks, use_b1, use_b2):
    """n_chunks: uniform message-chunk count per 128-dst tile (compile-time)."""
    CH = n_chunks                    # chunks per dst tile
    TOT_CH = NT * CH                 # total chunks per core
    M = TOT_CH * 128                 # total (padded) messages per core

    nc = bacc.Bacc(None)
    # ---- inputs (per core) ----
    xT = nc.dram_tensor("xT", [F, NPAD], BF16, kind="ExternalInput")
    w1 = nc.dram_tensor("w1", [F, F], BF16, kind="ExternalInput")
    w2 = nc.dram_tensor("w2", [F, F], BF16, kind="ExternalInput")
    wlin = nc.dram_tensor("wlin", [F, OUT_F], BF16, kind="ExternalInput")
    smat = nc.dram_tensor("smat", [128, TOT_CH * 128], BF16, kind="ExternalInput")
    gidx = nc.dram_tensor("gidx", [128, M // 16], mybir.dt.int16, kind="ExternalInput")
    pmask = nc.dram_tensor("pmask", [NPAD, G], BF16, kind="ExternalInput")
    if use_b1:
        b1b = nc.dram_tensor("b1b", [128, F], F32, kind="ExternalInput")
    if use_b2:
        b2b = nc.dram_tensor("b2b", [128, F], F32, kind="ExternalInput")
    # ---- outputs ----
    pooledT_out = nc.dram_tensor("pooledT", [F, G], F32, kind="ExternalOutput")
    outT_out = nc.dram_tensor("outT", [OUT_F, G], F32, kind="ExternalOutput")
    DBG = bool(os.environ.get("BASSDBG"))
    if DBG:
        dbg_out = nc.dram_tensor("dbg", [NPAD, F], F32, kind="ExternalOutput")
    # ---- internal DRAM ----
    xw1_loc = nc.dram_tensor("xw1_loc", [NPAD, F], BF16)
    xw1_full = nc.dram_tensor("xw1_full", [NPAD * NCORES, F], BF16,
                              addr_space="Shared")
    xw2_loc = nc.dram_tensor("xw2_loc", [NPAD, F], BF16)
    xw2_full = nc.dram_tensor("xw2_full", [NPAD * NCORES, F], BF16,
                              addr_space="Shared")
    ar_in = nc.dram_tensor("ar_in", [F, G], F32)
    ar_out = nc.dram_tensor("ar_out", [F, G], F32, addr_space="Shared")
    rg = [list(range(NCORES))]

    with tile.TileContext(nc) as tc:
        with (
            tc.tile_pool(name="const", bufs=1) as cpool,
            tc.tile_pool(name="sbuf", bufs=4) as sbuf,
            tc.tile_pool(name="psum", bufs=2, space="PSUM") as psum,
        ):
            # ---- resident constants ----
            xT_t = cpool.tile([128, 4, NPAD], BF16)
            nc.sync.dma_start(out=xT_t[:], in_=xT.rearrange("(a p) n -> p a n", p=128))
            w1_t = cpool.tile([128, 4, F], BF16)
            nc.sync.dma_start(out=w1_t[:], in_=w1.rearrange("(a p) n -> p a n", p=128))
            w2_t = cpool.tile([128, 4, F], BF16)
            nc.sync.dma_start(out=w2_t[:], in_=w2.rearrange("(a p) n -> p a n", p=128))
            wlin_t = cpool.tile([128, 4, OUT_F], BF16)
            nc.sync.dma_start(out=wlin_t[:], in_=wlin.rearrange("(a p) n -> p a n", p=128))
            s_t = cpool.tile([128, TOT_CH, 128], BF16)
            nc.sync.dma_start(
                out=s_t[:], in_=smat.rearrange("p (c d) -> p c d", d=128))
            gidx_t = cpool.tile([128, M // 16], mybir.dt.int16)
            nc.sync.dma_start(out=gidx_t[:], in_=gidx[:])
            pm_t = cpool.tile([128, NT, G], BF16)
            nc.sync.dma_start(out=pm_t[:], in_=pmask.rearrange("(t p) g -> p t g", p=128))
            ident = cpool.tile([128, 128], F32)
            make_identity(nc, ident[:])
            if use_b1:
                b1_t = cpool.tile([128, F], F32)
                nc.sync.dma_start(out=b1_t[:], in_=b1b[:])
            if use_b2:
                b2_t = cpool.tile([128, F], F32)
                nc.sync.dma_start(out=b2_t[:], in_=b2b[:])

            # ---- phase A: XW1 = x_c @ W1, per node tile ----
            for t in range(NT):
                ps = psum.tile([128, F], F32, tag="mm")
                for kc in range(4):
                    nc.tensor.matmul(
                        out=ps[:],
                        lhsT=xT_t[:, kc, t * 128:(t + 1) * 128],
                        rhs=w1_t[:, kc, :],
                        start=(kc == 0), stop=(kc == 3))
                xw_bf = sbuf.tile([128, F], BF16, tag="xw")
                nc.vector.tensor_copy(out=xw_bf[:], in_=ps[:])
                nc.sync.dma_start(out=xw1_loc[t * 128:(t + 1) * 128, :], in_=xw_bf[:])

            nc.gpsimd.collective_compute(
                "AllGather", mybir.AluOpType.bypass, replica_groups=rg,
                ins=[xw1_loc[:]], outs=[xw1_full[:]])

            # ---- layer 1 A-mult + transpose + XW2 per dst tile ----
            for t in range(NT):
                msg = sbuf.tile([128, CH, F], BF16, tag="msg")
                H1N = (CH // 2) * 128
                H2N = (CH - CH // 2) * 128
                nc.gpsimd.dma_gather(
                    out_ap=msg[:, :CH // 2, :], in_ap=xw1_full[:],
                    idxs_ap=gidx_t[:, t * (CH * 8):t * (CH * 8) + H1N // 16],
                    num_idxs=H1N, num_idxs_reg=H1N, elem_size=F,
                    single_packet=False)
                nc.gpsimd.dma_gather(
                    out_ap=msg[:, CH // 2:, :], in_ap=xw1_full[:],
                    idxs_ap=gidx_t[:, t * (CH * 8) + H1N // 16:(t + 1) * (CH * 8)],
                    num_idxs=H2N, num_idxs_reg=H2N, elem_size=F,
                    single_packet=False)
                ps = psum.tile([128, F], F32, tag="amult")
                for c in range(CH):
                    nc.tensor.matmul(
                        out=ps[:], lhsT=s_t[:, t * CH + c, :], rhs=msg[:, c, :],
                        start=(c == 0), stop=(c == CH - 1))
                if use_b1:
                    nc.vector.tensor_tensor(
                        out=ps[:], in0=ps[:], in1=b1_t[:], op=mybir.AluOpType.add)
                h1 = sbuf.tile([128, F], F32, tag="h1")
                nc.vector.tensor_scalar_max(h1[:], ps[:], 0.0)
                # transpose h1 -> h1T (4x [128,128], PE transpose in f32)
                h1T = sbuf.tile([128, 4, 128], BF16, tag="h1T")
                for fc in range(4):
                    pst = psum.tile([128, 128], F32, tag="tr")
                    nc.tensor.transpose(
                        out=pst[:], in_=h1[:, fc * 128:(fc + 1) * 128],
                        identity=ident[:])
                    nc.scalar.activation(
                        out=h1T[:, fc, :], in_=pst[:],
                        func=mybir.ActivationFunctionType.Copy)
                # XW2 tile = h1 @ W2
                ps2 = psum.tile([128, F], F32, tag="mm")
                for kc in range(4):
                    nc.tensor.matmul(
                        out=ps2[:], lhsT=h1T[:, kc, :], rhs=w2_t[:, kc, :],
                        start=(kc == 0), stop=(kc == 3))
                xw2_bf = sbuf.tile([128, F], BF16, tag="xw")
                nc.vector.tensor_copy(out=xw2_bf[:], in_=ps2[:])
                nc.sync.dma_start(out=xw2_loc[t * 128:(t + 1) * 128, :], in_=xw2_bf[:])

            nc.gpsimd.collective_compute(
                "AllGather", mybir.AluOpType.bypass, replica_groups=rg,
                ins=[xw2_loc[:]], outs=[xw2_full[:]])

            # ---- layer 2 A-mult + pooling ----
            pooled_acc = cpool.tile([128, 4, G], F32)
            nc.vector.memset(pooled_acc[:], 0.0)
            for t in range(NT):
                msg2 = sbuf.tile([128, CH, F], BF16, tag="msg")
                nc.gpsimd.dma_gather(
                    out_ap=msg2[:, :CH // 2, :], in_ap=xw2_full[:],
                    idxs_ap=gidx_t[:, t * (CH * 8):t * (CH * 8) + H1N // 16],
                    num_idxs=H1N, num_idxs_reg=H1N, elem_size=F,
                    single_packet=False)
                nc.gpsimd.dma_gather(
                    out_ap=msg2[:, CH // 2:, :], in_ap=xw2_full[:],
                    idxs_ap=gidx_t[:, t * (CH * 8) + H1N // 16:(t + 1) * (CH * 8)],
                    num_idxs=H2N, num_idxs_reg=H2N, elem_size=F,
                    single_packet=False)
                ps3 = psum.tile([128, F], F32, tag="amult")
                for c in range(CH):
                    nc.tensor.matmul(
                        out=ps3[:], lhsT=s_t[:, t * CH + c, :], rhs=msg2[:, c, :],
                        start=(c == 0), stop=(c == CH - 1))
                if use_b2:
                    nc.vector.tensor_tensor(
                        out=ps3[:], in0=ps3[:], in1=b2_t[:], op=mybir.AluOpType.add)
                h2 = sbuf.tile([128, F], BF16, tag="h2")
                nc.vector.tensor_scalar_max(h2[:], ps3[:], 0.0)
                if DBG:
                    h2f = sbuf.tile([128, F], F32, tag="h2f")
                    nc.vector.tensor_scalar_max(h2f[:], ps3[:], 0.0)
                    nc.sync.dma_start(
                        out=dbg_out[t * 128:(t + 1) * 128, :], in_=h2f[:])
                pool_ps = psum.tile([128, 4, G], F32, tag="pool")
                for fc in range(4):
                    nc.tensor.matmul(
                        out=pool_ps[:, fc, :],
                        lhsT=h2[:, fc * 128:(fc + 1) * 128],
                        rhs=pm_t[:, t, :],
                        start=True, stop=True)
                nc.vector.tensor_tensor(
                    out=pooled_acc[:], in0=pooled_acc[:], in1=pool_ps[:],
                    op=mybir.AluOpType.add)

            # ---- pooled AllReduce + final linear ----
            nc.sync.dma_start(
                out=ar_in.rearrange("(a p) g -> p a g", p=128), in_=pooled_acc[:])
            nc.gpsimd.collective_compute(
                "AllReduce", mybir.AluOpType.add, replica_groups=rg,
                ins=[ar_in[:]], outs=[ar_out[:]])
            nc.sync.dma_start(out=pooledT_out[:], in_=ar_out[:])
            prT = sbuf.tile([128, 4, G], F32, tag="prT")
            nc.sync.dma_start(
                out=prT[:], in_=ar_out.rearrange("(a p) g -> p a g", p=128))
            prT_bf = sbuf.tile([128, 4, G], BF16, tag="prT_bf")
            nc.vector.tensor_copy(out=prT_bf[:], in_=prT[:])
            ps4 = psum.tile([128, G], F32, tag="mm")
            for kc in range(4):
                nc.tensor.matmul(
                    out=ps4[:], lhsT=wlin_t[:, kc, :], rhs=prT_bf[:, kc, :],
                    start=(kc == 0), stop=(kc == 3))
            outT_sb = sbuf.tile([128, G], F32, tag="outT")
            nc.vector.tensor_copy(out=outT_sb[:], in_=ps4[:])
            nc.sync.dma_start(out=outT_out[:], in_=outT_sb[:])

    nc.compile()
    _fix_drain_waits(nc)
    return nc


def _prep_inputs(x, W1, b1, W2, b2, Wlin, blin, edge_index, batch):
    """Host-side sharding + index/norm prep. Returns (in_maps, n_chunks)."""
    src = np.asarray(edge_index[0], dtype=np.int64)
    dst = np.asarray(edge_index[1], dtype=np.int64)
    batch = np.asarray(batch, dtype=np.int64)
    x = np.asarray(x, dtype=np.float32)

    deg = np.bincount(dst, minlength=N).astype(np.float32) + 1.0
    dinv = (1.0 / np.sqrt(deg)).astype(np.float32)
    loop = np.arange(N, dtype=np.int64)
    msrc = np.concatenate([src, loop])
    mdst = np.concatenate([dst, loop])
    mnorm = dinv[msrc] * dinv[mdst]

    # gather-table row mapping (tables are per-core 1280-padded, concatenated)
    gmap = (msrc // NPC) * NPAD + (msrc % NPC)

    core_of = mdst // NPC
    per_core = []
    n_chunks = 1
    for c in range(NCORES):
        m = core_of == c
        s_c = gmap[m]
        d_c = (mdst[m] - c * NPC).astype(np.int64)
        n_c = mnorm[m]
        order = np.argsort(d_c, kind="stable")
        s_c, d_c, n_c = s_c[order], d_c[order], n_c[order]
        tile_id = d_c // 128
        counts = np.bincount(tile_id, minlength=NT)
        n_chunks = max(n_chunks, int(np.max((counts + 127) // 128)))
        per_core.append((s_c, d_c, n_c, tile_id, counts))

    CH = n_chunks
    TOT_CH = NT * CH
    M = TOT_CH * 128

    bf = ml_dtypes.bfloat16
    w1_b = np.asarray(W1, dtype=np.float32).astype(bf)
    w2_b = np.asarray(W2, dtype=np.float32).astype(bf)
    wlin_b = np.asarray(Wlin, dtype=np.float32).astype(bf)
    use_b1 = bool(np.any(np.asarray(b1)))
    use_b2 = bool(np.any(np.asarray(b2)))

    in_maps = []
    for c in range(NCORES):
        s_c, d_c, n_c, tile_id, counts = per_core[c]
        # flat padded message arrays
        idx_flat = np.zeros(M, dtype=np.int16)
        s_host = np.zeros((128, TOT_CH, 128), dtype=np.float32)
        pos = 0
        for t in range(NT):
            cnt = int(counts[t])
            base = t * CH * 128
            sl = slice(pos, pos + cnt)
            idx_flat[base:base + cnt] = s_c[sl].astype(np.int16)
            j = np.arange(cnt)
            s_host[j % 128, t * CH + j // 128, d_c[sl] % 128] = n_c[sl]
            pos += cnt
        idx_wrapped = np.tile(idx_flat.reshape(-1, 16).T, (8, 1)).copy()

        xpad = np.zeros((NPAD, F), dtype=np.float32)
        xpad[:NPC] = x[c * NPC:(c + 1) * NPC]
        xT_b = np.ascontiguousarray(xpad.T).astype(bf)

        pm = np.zeros((NPAD, G), dtype=np.float32)
        pm[np.arange(NPC), batch[c * NPC:(c + 1) * NPC]] = 1.0

        im = {
            "xT": xT_b,
            "w1": w1_b, "w2": w2_b, "wlin": wlin_b,
            "smat": s_host.reshape(128, TOT_CH * 128).astype(bf),
            "gidx": idx_wrapped,
            "pmask": pm.astype(bf),
        }
        if use_b1:
            im["b1b"] = np.broadcast_to(
                np.asarray(b1, dtype=np.float32), (128, F)).copy()
        if use_b2:
            im["b2b"] = np.broadcast_to(
                np.asarray(b2, dtype=np.float32), (128, F)).copy()
        in_maps.append(im)
    return in_maps, CH, use_b1, use_b2


_CACHE = {}


def kernel(x, W1, b1, W2, b2, Wlin, blin, edge_index, batch, _trace=False):
    in_maps, CH, use_b1, use_b2 = _prep_inputs(
        x, W1, b1, W2, b2, Wlin, blin, edge_index, batch)
    key = (CH, use_b1, use_b2)
    if key not in _CACHE:
        _CACHE[key] = build(CH, use_b1, use_b2)
    nc = _CACHE[key]
    res = run_bass_kernel_spmd(nc, in_maps, list(range(NCORES)), trace=_trace)
    r0 = res.results[0]
    pooled = np.ascontiguousarray(r0["pooledT"].T).astype(np.float32)
    out = np.ascontiguousarray(r0["outT"].T).astype(np.float32)
    out = out + np.asarray(blin, dtype=np.float32)[None, :]
    kernel._last_exec_time_ns = res.exec_time_ns
    kernel._last_results = res
    return (pooled, out)


# revision 13
# speedup vs baseline: 1.1584x; 1.1584x over previous
"""GCN 2-layer + global_add_pool + linear on 8 TRN2 NeuronCores.

Strategy (data-parallel over dst-node ranges):
  - core c owns dst nodes [c*1250, (c+1)*1250)
  - host precomputes, per core: gather indices (edge srcs sorted by dst,
    padded per 128-dst tile to a uniform chunk count) and a block one-hot
    "segment sum" matrix S with the GCN edge norms as values.
  - device: XW1 = x_c @ W1 (bf16 PE) -> AllGather -> dma_gather messages ->
    S-matmul segmented sum (PE, accumulating in PSUM f32) -> relu -> H1;
    transpose H1 tiles on PE -> XW2 = H1 @ W2 -> AllGather -> gather ->
    S-matmul -> relu -> H2; pooling via per-tile graph-mask matmul ->
    AllReduce -> out^T = Wlin^T @ pooled^T.
  - biases are zeros in this problem's setup; b1/b2 get a DVE add only if
    nonzero, blin is added on the host (linear post-op).
"""
import os
import numpy as np
import ml_dtypes

import concourse.bass as bass
import concourse.mybir as mybir
import concourse.tile as tile
from concourse import bacc
from concourse.bass_utils import run_bass_kernel_spmd
from concourse.masks import make_identity

N = 10000
E = 160000
F = 512          # IN_F == CL1 == CL2 == 512
OUT_F = 128
G = 64
NCORES = 8
NPC = N // NCORES          # 1250 nodes per core
NPAD = 1280                # padded to 10 tiles of 128
NT = NPAD // 128           # 10 node tiles per core
BF16 = mybir.dt.bfloat16
F32 = mybir.dt.float32


def _fix_drain_waits(nc, max_waits=1):
    """walrus CTRL instrs (Drain) accept only 1 sync wait; split extras."""
    for fn in nc.m.functions:
        for bb in fn.blocks:
            insts = bb.instructions
            i = 0
            while i < len(insts):
                inst = insts[i]
                si = inst.sync_info
                if si is not None and si.on_wait and len(si.on_wait) > max_waits:
                    waits = list(si.on_wait)
                    chunks = [waits[j:j + max_waits]
                              for j in range(0, len(waits), max_waits)]
                    inst.sync_info = mybir.SyncInfo(
                        on_wait=chunks[-1], on_update=list(si.on_update))
                    for k, ch in enumerate(chunks[:-1]):
                        d = mybir.InstDrain(
                            name=f"{inst.name}_ws{k}",
                            engine=inst.engine,
                            ins=[], outs=[],
                            sync_info=mybir.SyncInfo(on_wait=ch, on_update=[]),
                        )
                        insts.insert(i + k, d)
                    i += len(chunks) - 1
                i += 1


def build(n_chunks, use_b1, use_b2):
    """n_chunks: uniform message-chunk count per 128-dst tile (compile-time)."""
    CH = n_chunks                    # chunks per dst tile
    TOT_CH = NT * CH                 # total chunks per core
    M = TOT_CH * 128                 # total (padded) messages per core

    nc = bacc.Bacc(None)
    # ---- inputs (per core) ----
    xT = nc.dram_tensor("xT", [F, NPAD], BF16, kind="ExternalInput")
    w1 = nc.dram_tensor("w1", [F, F], BF16, kind="ExternalInput")
    w2 = nc.dram_tensor("w2", [F, F], BF16, kind="ExternalInput")
    wlin = nc.dram_tensor("wlin", [F, OUT_F], BF16, kind="ExternalInput")
    smat = nc.dram_tensor("smat", [128, TOT_CH * 128], BF16, kind="ExternalInput")
    gidx = nc.dram_tensor("gidx", [128, M // 16], mybir.dt.int16, kind="ExternalInput")
    pmask = nc.dram_tensor("pmask", [NPAD, G], BF16, kind="ExternalInput")
    if use_b1:
        b1b = nc.dram_tensor("b1b", [128, F], F32, kind="ExternalInput")
    if use_b2:
        b2b = nc.dram_tensor("b2b", [128, F], F32, kind="ExternalInput")
    # ---- outputs ----
    pooledT_out = nc.dram_tensor("pooledT", [F, G], F32, kind="ExternalOutput")
    outT_out = nc.dram_tensor("outT", [OUT_F, G], F32, kind="ExternalOutput")
    DBG = bool(os.environ.get("BASSDBG"))
    if DBG:
        dbg_out = nc.dram_tensor("dbg", [NPAD, F], F32, kind="ExternalOutput")
    # ---- internal DRAM ----
    xw1_loc = nc.dram_tensor("xw1_loc", [NPAD, F], BF16)
    xw1_full = nc.dram_tensor("xw1_full", [NPAD * NCORES, F], BF16,
                              addr_space="Shared")
    xw2_loc = nc.dram_tensor("xw2_loc", [NPAD, F], BF16)
    xw2_full = nc.dram_tensor("xw2_full", [NPAD * NCORES, F], BF16,
                              addr_space="Shared")
    ar_in = nc.dram_tensor("ar_in", [F, G], F32)
    ar_out = nc.dram_tensor("ar_out", [F, G], F32, addr_space="Shared")
    rg = [list(range(NCORES))]

    with tile.TileContext(nc) as tc:
        with (
            tc.tile_pool(name="const", bufs=1) as cpool,
            tc.tile_pool(name="sbuf", bufs=4) as sbuf,
            tc.tile_pool(name="psum", bufs=2, space="PSUM") as psum,
        ):
            # ---- resident constants ----
            xT_t = cpool.tile([128, 4, NPAD], BF16)
            nc.sync.dma_start(out=xT_t[:], in_=xT.rearrange("(a p) n -> p a n", p=128))
            w1_t = cpool.tile([128, 4, F], BF16)
            nc.sync.dma_start(out=w1_t[:], in_=w1.rearrange("(a p) n -> p a n", p=128))
            w2_t = cpool.tile([128, 4, F], BF16)
            nc.sync.dma_start(out=w2_t[:], in_=w2.rearrange("(a p) n -> p a n", p=128))
            wlin_t = cpool.tile([128, 4, OUT_F], BF16)
            nc.sync.dma_start(out=wlin_t[:], in_=wlin.rearrange("(a p) n -> p a n", p=128))
            s_t = cpool.tile([128, TOT_CH, 128], BF16)
            nc.sync.dma_start(
                out=s_t[:], in_=smat.rearrange("p (c d) -> p c d", d=128))
            gidx_t = cpool.tile([128, M // 16], mybir.dt.int16)
            nc.sync.dma_start(out=gidx_t[:], in_=gidx[:])
            pm_t = cpool.tile([128, NT, G], BF16)
            nc.sync.dma_start(out=pm_t[:], in_=pmask.rearrange("(t p) g -> p t g", p=128))
            ident = cpool.tile([128, 128], F32)
            make_identity(nc, ident[:])
            if use_b1:
                b1_t = cpool.tile([128, F], F32)
                nc.sync.dma_start(out=b1_t[:], in_=b1b[:])
            if use_b2:
                b2_t = cpool.tile([128, F], F32)
                nc.sync.dma_start(out=b2_t[:], in_=b2b[:])

            # ---- phase A: XW1 = x_c @ W1, per node tile ----
            for t in range(NT):
                ps = psum.tile([128, F], F32, tag="mm")
                for kc in range(4):
                    nc.tensor.matmul(
                        out=ps[:],
                        lhsT=xT_t[:, kc, t * 128:(t + 1) * 128],
                        rhs=w1_t[:, kc, :],
                        start=(kc == 0), stop=(kc == 3))
                xw_bf = sbuf.tile([128, F], BF16, tag="xw")
                nc.vector.tensor_copy(out=xw_bf[:], in_=ps[:])
                nc.sync.dma_start(out=xw1_loc[t * 128:(t + 1) * 128, :], in_=xw_bf[:])

            nc.gpsimd.collective_compute(
                "AllGather", mybir.AluOpType.bypass, replica_groups=rg,
                ins=[xw1_loc[:]], outs=[xw1_full[:]])

            # ---- layer 1 A-mult + transpose + XW2 per dst tile ----
            for t in range(NT):
                msg = sbuf.tile([128, CH, F], BF16, tag="msg")
                nc.gpsimd.dma_gather(
                    out_ap=msg[:], in_ap=xw1_full[:],
                    idxs_ap=gidx_t[:, t * (CH * 8):(t + 1) * (CH * 8)],
                    num_idxs=CH * 128, num_idxs_reg=CH * 128, elem_size=F,
                    single_packet=False)
                ps = psum.tile([128, F], F32, tag="amult")
                for c in range(CH):
                    nc.tensor.matmul(
                        out=ps[:], lhsT=s_t[:, t * CH + c, :], rhs=msg[:, c, :],
                        start=(c == 0), stop=(c == CH - 1))
                if use_b1:
                    nc.vector.tensor_tensor(
                        out=ps[:], in0=ps[:], in1=b1_t[:], op=mybir.AluOpType.add)
                h1 = sbuf.tile([128, F], F32, tag="h1")
                nc.vector.tensor_scalar_max(h1[:], ps[:], 0.0)
                # transpose h1 -> h1T (4x [128,128], PE transpose in f32)
                h1T = sbuf.tile([128, 4, 128], BF16, tag="h1T")
                for fc in range(4):
                    pst = psum.tile([128, 128], F32, tag="tr")
                    nc.tensor.transpose(
                        out=pst[:], in_=h1[:, fc * 128:(fc + 1) * 128],
                        identity=ident[:])
                    nc.scalar.activation(
                        out=h1T[:, fc, :], in_=pst[:],
                        func=mybir.ActivationFunctionType.Copy)
                # XW2 tile = h1 @ W2
                ps2 = psum.tile([128, F], F32, tag="mm")
                for kc in range(4):
                    nc.tensor.matmul(
                        out=ps2[:], lhsT=h1T[:, kc, :], rhs=w2_t[:, kc, :],
                        start=(kc == 0), stop=(kc == 3))
                xw2_bf = sbuf.tile([128, F], BF16, tag="xw")
                nc.vector.tensor_copy(out=xw2_bf[:], in_=ps2[:])
                nc.sync.dma_start(out=xw2_loc[t * 128:(t + 1) * 128, :], in_=xw2_bf[:])

            nc.gpsimd.collective_compute(
                "AllGather", mybir.AluOpType.bypass, replica_groups=rg,
                ins=[xw2_loc[:]], outs=[xw2_full[:]])

            # ---- layer 2 A-mult + pooling ----
            pooled_acc = cpool.tile([128, 4, G], F32)
            nc.vector.memset(pooled_acc[:], 0.0)
            for t in range(NT):
                msg2 = sbuf.tile([128, CH, F], BF16, tag="msg")
                nc.gpsimd.dma_gather(
                    out_ap=msg2[:], in_ap=xw2_full[:],
                    idxs_ap=gidx_t[:, t * (CH * 8):(t + 1) * (CH * 8)],
                    num_idxs=CH * 128, num_idxs_reg=CH * 128, elem_size=F,
                    single_packet=False)
                ps3 = psum.tile([128, F], F32, tag="amult")
                for c in range(CH):
                    nc.tensor.matmul(
                        out=ps3[:], lhsT=s_t[:, t * CH + c, :], rhs=msg2[:, c, :],
                        start=(c == 0), stop=(c == CH - 1))
                if use_b2:
                    nc.vector.tensor_tensor(
                        out=ps3[:], in0=ps3[:], in1=b2_t[:], op=mybir.AluOpType.add)
                h2 = sbuf.tile([128, F], BF16, tag="h2")
                nc.vector.tensor_scalar_max(h2[:], ps3[:], 0.0)
                if DBG:
                    h2f = sbuf.tile([128, F], F32, tag="h2f")
                    nc.vector.tensor_scalar_max(h2f[:], ps3[:], 0.0)
                    nc.sync.dma_start(
                        out=dbg_out[t * 128:(t + 1) * 128, :], in_=h2f[:])
                pool_ps = psum.tile([128, 4, G], F32, tag="pool")
                for fc in range(4):
                    nc.tensor.matmul(
                        out=pool_ps[:, fc, :],
                        lhsT=h2[:, fc * 128:(fc + 1) * 128],
                        rhs=pm_t[:, t, :],
                        start=True, stop=True)
                nc.vector.tensor_tensor(
                    out=pooled_acc[:], in0=pooled_acc[:], in1=pool_ps[:],
                    op=mybir.AluOpType.add)

            # ---- pooled AllReduce + final linear ----
            nc.sync.dma_start(
                out=ar_in.rearrange("(a p) g -> p a g", p=128), in_=pooled_acc[:])
            nc.gpsimd.collective_compute(
                "AllReduce", mybir.AluOpType.add, replica_groups=rg,
                ins=[ar_in[:]], outs=[ar_out[:]])
            nc.sync.dma_start(out=pooledT_out[:], in_=ar_out[:])
            prT = sbuf.tile([128, 4, G], F32, tag="prT")
            nc.sync.dma_start(
                out=prT[:], in_=ar_out.rearrange("(a p) g -> p a g", p=128))
            prT_bf = sbuf.tile([128, 4, G], BF16, tag="prT_bf")
            nc.vector.tensor_copy(out=prT_bf[:], in_=prT[:])
            ps4 = psum.tile([128, G], F32, tag="mm")
            for kc in range(4):
                nc.tensor.matmul(
                    out=ps4[:], lhsT=wlin_t[:, kc, :], rhs=prT_bf[:, kc, :],
                    start=(kc == 0), stop=(kc == 3))
            outT_sb = sbuf.tile([128, G], F32, tag="outT")
            nc.vector.tensor_copy(out=outT_sb[:], in_=ps4[:])
            nc.sync.dma_start(out=outT_out[:], in_=outT_sb[:])

    nc.compile()
    _fix_drain_waits(nc)
    return nc


def _prep_inputs(x, W1, b1, W2, b2, Wlin, blin, edge_index, batch):
    """Host-side sharding + index/norm prep. Returns (in_maps, n_chunks)."""
    src = np.asarray(edge_index[0], dtype=np.int64)
    dst = np.asarray(edge_index[1], dtype=np.int64)
    batch = np.asarray(batch, dtype=np.int64)
    x = np.asarray(x, dtype=np.float32)

    deg = np.bincount(dst, minlength=N).astype(np.float32) + 1.0
    dinv = (1.0 / np.sqrt(deg)).astype(np.float32)
    loop = np.arange(N, dtype=np.int64)
    msrc = np.concatenate([src, loop])
    mdst = np.concatenate([dst, loop])
    mnorm = dinv[msrc] * dinv[mdst]

    # gather-table row mapping (tables are per-core 1280-padded, concatenated)
    gmap = (msrc // NPC) * NPAD + (msrc % NPC)

    core_of = mdst // NPC
    per_core = []
    n_chunks = 1
    for c in range(NCORES):
        m = core_of == c
        s_c = gmap[m]
        d_c = (mdst[m] - c * NPC).astype(np.int64)
        n_c = mnorm[m]
        order = np.argsort(d_c, kind="stable")
        s_c, d_c, n_c = s_c[order], d_c[order], n_c[order]
        tile_id = d_c // 128
        counts = np.bincount(tile_id, minlength=NT)
        n_chunks = max(n_chunks, int(np.max((counts + 127) // 128)))
        per_core.append((s_c, d_c, n_c, tile_id, counts))

    CH = n_chunks
    TOT_CH = NT * CH
    M = TOT_CH * 128

    bf = ml_dtypes.bfloat16
    w1_b = np.asarray(W1, dtype=np.float32).astype(bf)
    w2_b = np.asarray(W2, dtype=np.float32).astype(bf)
    wlin_b = np.asarray(Wlin, dtype=np.float32).astype(bf)
    use_b1 = bool(np.any(np.asarray(b1)))
    use_b2 = bool(np.any(np.asarray(b2)))

    in_maps = []
    for c in range(NCORES):
        s_c, d_c, n_c, tile_id, counts = per_core[c]
        # flat padded message arrays
        idx_flat = np.zeros(M, dtype=np.int16)
        s_host = np.zeros((128, TOT_CH, 128), dtype=np.float32)
        pos = 0
        for t in range(NT):
            cnt = int(counts[t])
            base = t * CH * 128
            sl = slice(pos, pos + cnt)
            idx_flat[base:base + cnt] = s_c[sl].astype(np.int16)
            j = np.arange(cnt)
            s_host[j % 128, t * CH + j // 128, d_c[sl] % 128] = n_c[sl]
            pos += cnt
        idx_wrapped = np.tile(idx_flat.reshape(-1, 16).T, (8, 1)).copy()

        xpad = np.zeros((NPAD, F), dtype=np.float32)
        xpad[:NPC] = x[c * NPC:(c + 1) * NPC]
        xT_b = np.ascontiguousarray(xpad.T).astype(bf)

        pm = np.zeros((NPAD, G), dtype=np.float32)
        pm[np.arange(NPC), batch[c * NPC:(c + 1) * NPC]] = 1.0

        im = {
            "xT": xT_b,
            "w1": w1_b, "w2": w2_b, "wlin": wlin_b,
            "smat": s_host.reshape(128, TOT_CH * 128).astype(bf),
            "gidx": idx_wrapped,
            "pmask": pm.astype(bf),
        }
        if use_b1:
            im["b1b"] = np.broadcast_to(
                np.asarray(b1, dtype=np.float32), (128, F)).copy()
        if use_b2:
            im["b2b"] = np.broadcast_to(
                np.asarray(b2, dtype=np.float32), (128, F)).copy()
        in_maps.append(im)
    return in_maps, CH, use_b1, use_b2


_CACHE = {}


def kernel(x, W1, b1, W2, b2, Wlin, blin, edge_index, batch, _trace=False):
    in_maps, CH, use_b1, use_b2 = _prep_inputs(
        x, W1, b1, W2, b2, Wlin, blin, edge_index, batch)
    key = (CH, use_b1, use_b2)
    if key not in _CACHE:
        _CACHE[key] = build(CH, use_b1, use_b2)
    nc = _CACHE[key]
    res = run_bass_kernel_spmd(nc, in_maps, list(range(NCORES)), trace=_trace)
    r0 = res.results[0]
    pooled = np.ascontiguousarray(r0["pooledT"].T).astype(np.float32)
    out = np.ascontiguousarray(r0["outT"].T).astype(np.float32)
    out = out + np.asarray(blin, dtype=np.float32)[None, :]
    kernel._last_exec_time_ns = res.exec_time_ns
    kernel._last_results = res
    return (pooled, out)
